# revision 1
# baseline (speedup 1.0000x reference)
"""Trainium2 Bass kernel for nn_CausalSelfAttention_40810779247124.

Head-sharded (tensor-parallel) causal self-attention prefill across 8
NeuronCores: 2 heads per core. All SBUF operands are bf16 (psum f32),
which keeps every matmul at 1 cycle/row (f32r pays 4x below 256-wide
moving dims), halves DMA bytes, and lands ~5e-3 relative error.

Per core:
  phase 1: QKV projection for its 2 heads. Q/K kept resident in SBUF as
           [e, tok] bf16; V produced PE-transposed as [tok, e] bf16.
  phase 2: attention with scores transposed: scT[t,s] = K^T Q. Causal
           work is exact at 128-row granularity: for the 4 diagonal
           t-chunks of each 512-query tile the matmul/exp/z/wv are
           narrowed to the s-columns that need them (17408 free-cycles
           per (batch,head) -- the causal minimum). exp on ACT,
           denominator z via ones-column matmul on PE (the fastest
           partition-reducer), 1/z broadcast across partitions on the
           idle GPSIMD engine.
  phase 3: out-projection partial for this core's d-slice, interleaved
           per 512-token step one step behind attention (hides the
           finalize chain), psum->sbuf copies alternate DVE/ACT, bf16
           partials DMA'd out; the all-reduce over cores is the host
           summing 8 partials.

The host verifies mask/cache_pos match causal prefill and falls back to
a numpy reference otherwise.
"""

import sys

sys.path.insert(0, "/opt/trn_rl_repo")

import numpy as np

B = 2
S = 2048
T = 4096
NS = 2048          # n_state
H = 16
DH = 128
NCORES = 8
HPC = H // NCORES  # heads per core = 2
DPC = HPC * DH     # d-slice per core = 256
TOK = B * S        # 4096 tokens across batches
SCALE = 1.0 / float(np.sqrt(DH))

_CACHED = {}


def _build_program():
    import concourse.bacc as bacc
    import concourse.bass as bass
    import concourse.tile as tile
    from concourse import mybir

    bf16 = mybir.dt.bfloat16
    f32 = mybir.dt.float32
    f32r = mybir.dt.float32r
    EXP = mybir.ActivationFunctionType.Exp
    COPY = mybir.ActivationFunctionType.Copy

    nc = bacc.Bacc()

    xT = nc.dram_tensor("xT", [NS, TOK], bf16, kind="ExternalInput")
    wT = nc.dram_tensor("wT", [NS, 6 * DH], bf16, kind="ExternalInput")
    woutT = nc.dram_tensor("woutT", [DPC, NS], bf16, kind="ExternalInput")
    cmask = nc.dram_tensor("cmask", [DH, DH], bf16, kind="ExternalInput")
    outp = nc.dram_tensor("outp", [TOK, NS], bf16, kind="ExternalOutput")

    NT = TOK // 512   # 8 tok-tiles of 512
    NK = NS // 128    # 16 contraction chunks

    with tile.TileContext(nc) as tc:
        with (
            tc.tile_pool(name="constp", bufs=1) as constp,
            tc.tile_pool(name="vresp", bufs=1) as vresp,
            tc.tile_pool(name="qkresp", bufs=1) as qkresp,
            tc.tile_pool(name="woutp", bufs=1) as woutp,
        ):
            # tri[t, s] = 1.0 if s >= t (within-diag causal mask).
            # Column 127 is all ones -> doubles as the z ones-column.
            tri = constp.tile([DH, DH], bf16)
            ones_col = tri[:, 127:128]

            # V resident: v_res[p, c, e] = V[c*128+p, e] (tok-major)
            v_res = vresp.tile([128, TOK // 128, DPC], bf16)
            # Q,K resident [e-block(q0,q1,k0,k1), tok]
            qk_res = qkresp.tile([128, 4, TOK], bf16)

            # ---------------- phase 1: QKV projection ----------------
            with (
                tc.tile_pool(name="wp", bufs=1) as wp,
                tc.tile_pool(name="xp", bufs=3) as xp,
                tc.tile_pool(name="qkv_ps", bufs=4, space="PSUM") as qkv_ps,
                tc.tile_pool(name="v_ps", bufs=4, space="PSUM") as v_ps,
            ):
                nc.scalar.dma_start(out=tri[:, :], in_=cmask[:, :])
                w_sb = wp.tile([128, NK, 6 * DH], bf16)
                wout_sb = woutp.tile([128, HPC, NS], bf16)

                for a in range(NT):
                    pss = [
                        qkv_ps.tile([128, 512], f32, tag="qkv", name=f"qkv{m}")
                        for m in range(4)
                    ]
                    vps = [
                        v_ps.tile([128, 256], f32, tag="vps", name=f"vps{t}")
                        for t in range(4)
                    ]
                    for half in range(2):
                        x_sb = xp.tile([128, NK // 2, 512], bf16, tag="x_sb")
                        for kc in range(NK // 2):
                            nc.sync.dma_start(
                                out=x_sb[:, kc, :],
                                in_=xT[
                                    1024 * half + 128 * kc : 1024 * half
                                    + 128 * (kc + 1),
                                    512 * a : 512 * (a + 1),
                                ],
                            )
                        for kc in range(NK // 2):
                            kk = half * (NK // 2) + kc
                            if a == 0:
                                if kk == 0:
                                    for mm in range(6):
                                        nc.scalar.dma_start(
                                            out=w_sb[
                                                :, kk, 128 * mm : 128 * (mm + 1)
                                            ],
                                            in_=wT[
                                                128 * kk : 128 * (kk + 1),
                                                128 * mm : 128 * (mm + 1),
                                            ],
                                        )
                                else:
                                    nc.scalar.dma_start(
                                        out=w_sb[:, kk, :],
                                        in_=wT[128 * kk : 128 * (kk + 1), :],
                                    )
                            for m in range(4):
                                nc.tensor.matmul(
                                    pss[m],
                                    w_sb[:, kk, 128 * m : 128 * (m + 1)],
                                    x_sb[:, kc, :],
                                    start=(kk == 0),
                                    stop=(kk == NK - 1),
                                )
                            for t in range(4):
                                nc.tensor.matmul(
                                    vps[t],
                                    x_sb[:, kc, 128 * t : 128 * (t + 1)],
                                    w_sb[:, kk, 512:768],
                                    start=(kk == 0),
                                    stop=(kk == NK - 1),
                                )
                    if a < NT - 1:
                        for m in range(4):
                            nc.vector.tensor_copy(
                                out=qk_res[:, m, 512 * a : 512 * (a + 1)],
                                in_=pss[m],
                            )
                        for t in range(4):
                            nc.vector.tensor_copy(
                                out=v_res[:, 4 * a + t, :], in_=vps[t]
                            )
                    else:
                        # Last tile: drain all 8 psum banks across three
                        # engines at once so phase 2 (which reuses these
                        # banks) isn't gated on a serial DVE copy chain.
                        for m in range(4):
                            dst = qk_res[:, m, 512 * a : 512 * (a + 1)]
                            if m % 2 == 0:
                                nc.vector.tensor_copy(out=dst, in_=pss[m])
                            else:
                                nc.scalar.activation(
                                    out=dst, in_=pss[m], func=COPY, scale=1.0
                                )
                        for t in range(4):
                            dst = v_res[:, 4 * a + t, :]
                            if t % 2 == 0:
                                nc.vector.tensor_copy(out=dst, in_=vps[t])
                            else:
                                nc.scalar.activation(
                                    out=dst, in_=vps[t], func=COPY, scale=1.0
                                )
                    if a == 0:
                        # Prefetch w_out and warm the ACT exp table while
                        # the PE grinds through the remaining QKV tiles.
                        for h in range(HPC):
                            nc.scalar.dma_start(
                                out=wout_sb[:, h, :],
                                in_=woutT[128 * h : 128 * (h + 1), :],
                            )
                        warm = constp.tile([1, 1], f32)
                        nc.scalar.activation(
                            out=warm, in_=tri[0:1, 0:1], func=EXP, scale=1.0
                        )

            # ------- phases 2+3: attention + out-projection, interleaved -------
            with (
                tc.tile_pool(name="ptp", bufs=8) as ptp,
                tc.tile_pool(name="zrp", bufs=2) as zrp,
                tc.tile_pool(name="zbp", bufs=2) as zbp,
                tc.tile_pool(name="wvnp", bufs=4) as wvnp,
                tc.tile_pool(name="ostage", bufs=3) as ostage,
                tc.tile_pool(name="sc_ps", bufs=3, space="PSUM") as sc_ps,
                tc.tile_pool(name="wv_ps", bufs=2, space="PSUM") as wv_ps,
                tc.tile_pool(name="z_ps", bufs=1, space="PSUM") as z_ps,
                tc.tile_pool(name="o_ps", bufs=2, space="PSUM") as o_ps,
            ):
                def attn_tile(b, h, ast, wvn):
                    q_sb = qk_res[:, h, S * b + 512 * ast : S * b + 512 * (ast + 1)]
                    wv = wv_ps.tile([128, 512], f32, tag="wv")
                    z = z_ps.tile([1, 512], f32, tag="z")
                    zr = zrp.tile([1, 512], f32r, tag="zr")
                    zbs = zbp.tile([128, 512], f32r, tag="zbs")
                    nfull = 4 * ast
                    nj = nfull + 4
                    for j in range(nj):
                        p = j - nfull
                        lo = 0 if p < 0 else 128 * p  # causal narrowing
                        sc = sc_ps.tile([128, 512], f32, tag="sc")
                        nc.tensor.matmul(
                            sc[:, lo:],
                            qk_res[:, 2 + h, S * b + 128 * j : S * b + 128 * (j + 1)],
                            q_sb[:, lo:],
                            start=True,
                            stop=True,
                        )
                        pt = ptp.tile([128, 512], bf16, tag="pt")
                        nc.scalar.activation(
                            out=pt[:, lo:], in_=sc[:, lo:], func=EXP, scale=SCALE
                        )
                        if p >= 0:
                            nc.gpsimd.tensor_mul(
                                pt[:, lo : lo + 128], pt[:, lo : lo + 128], tri
                            )
                        nc.tensor.matmul(
                            z[:, lo:],
                            ones_col,
                            pt[:, lo:],
                            start=(j == 0),
                            stop=(j == nj - 1),
                            skip_group_check=True,
                        )
                        nc.tensor.matmul(
                            wv[:, lo:],
                            v_res[:, 16 * b + j, 128 * h : 128 * (h + 1)],
                            pt[:, lo:],
                            start=(j == 0),
                            stop=(j == nj - 1),
                            skip_group_check=True,
                        )
                        if p >= 0:
                            # Columns [128p, 128p+128) of z and wv are final
                            # after diagonal chunk p: normalize them now so the
                            # finalize chain is hidden under later chunks.
                            cs = slice(lo, lo + 128)
                            with nc.allow_low_precision(
                                reason="f32r is bit-identical to f32"
                            ):
                                nc.vector.reciprocal(out=zr[:, cs], in_=z[:, cs])
                            nc.gpsimd.partition_broadcast(
                                zbs[:, cs], zr[:, cs], channels=128
                            )
                            nc.vector.tensor_mul(
                                wvn[:, 512 * ast + lo : 512 * ast + lo + 128],
                                wv[:, cs],
                                zbs[:, cs],
                            )

                def outproj(b, ast, wvn_pair, last=False):
                    for tk in range(4):
                        toff = 512 * ast + 128 * tk
                        ost = ostage.tile([128, NS], bf16, tag="ost")
                        for n in range(4):
                            # The last job runs with attention done: rotate
                            # through the idle sc banks too for extra depth.
                            if last and n % 2:
                                ops = sc_ps.tile([128, 512], f32, tag="sc")
                            else:
                                ops = o_ps.tile([128, 512], f32, tag="ops")
                            for h in range(HPC):
                                nc.tensor.matmul(
                                    ops,
                                    wvn_pair[h][:, toff : toff + 128],
                                    wout_sb[:, h, 512 * n : 512 * (n + 1)],
                                    start=(h == 0),
                                    stop=(h == HPC - 1),
                                )
                            # Alternate drain engines: exp (ACT) runs during
                            # attention, drains run during out-proj windows.
                            dst = ost[:, 512 * n : 512 * (n + 1)]
                            if n % 2 == 0:
                                nc.vector.tensor_copy(out=dst, in_=ops)
                            else:
                                nc.scalar.activation(
                                    out=dst, in_=ops, func=COPY, scale=1.0
                                )
                            if n % 2 == 1:
                                nc.sync.dma_start(
                                    out=outp[
                                        S * b + toff : S * b + toff + 128,
                                        1024 * (n // 2) : 1024 * (n // 2 + 1),
                                    ],
                                    in_=ost[:, 1024 * (n // 2) : 1024 * (n // 2 + 1)],
                                )

                jobs = []
                for b in range(B):
                    wvn_pair = [
                        wvnp.tile([128, S], bf16, tag="wvn", name=f"wvn{b}_{h}")
                        for h in range(HPC)
                    ]
                    for ast in range(4):
                        attn_tile(b, 0, ast, wvn_pair[0])
                        if jobs:
                            outproj(*jobs.pop(0))
                        attn_tile(b, 1, ast, wvn_pair[1])
                        jobs.append((b, ast, wvn_pair))
                while jobs:
                    outproj(*jobs.pop(0), last=True)

    nc.compile()
    return nc


def _causal_fastpath_ok(mask, cache_pos):
    if cache_pos.shape != (S,) or not np.array_equal(
        np.asarray(cache_pos), np.arange(S, dtype=np.int64).astype(cache_pos.dtype)
    ):
        return False
    m = np.asarray(mask).reshape(S, T)
    rows = np.arange(S)[:, None]
    cols = np.arange(T)[None, :]
    return np.array_equal(m, cols <= rows)


def _numpy_fallback(input_ids, mask, cache_pos, w_qkv, w_out, k_cache, v_cache):
    x = np.asarray(input_ids, dtype=np.float32)
    qkv = np.einsum("bsd,ed->bse", x, np.asarray(w_qkv, np.float32))
    q, k, v = np.split(qkv, 3, axis=-1)

    def heads(t):
        return t.reshape(B, S, H, DH).transpose(0, 2, 1, 3)

    q, k, v = heads(q), heads(k), heads(v)
    kf = np.array(k_cache, np.float32)
    vf = np.array(v_cache, np.float32)
    kf[:, :, np.asarray(cache_pos)] = k
    vf[:, :, np.asarray(cache_pos)] = v
    sc = np.einsum("bhsd,bhtd->bhst", q, kf) * SCALE
    sc = np.where(np.asarray(mask), sc, np.finfo(np.float32).min)
    sc = sc - sc.max(axis=-1, keepdims=True)
    p = np.exp(sc)
    p = p / p.sum(axis=-1, keepdims=True)
    wv = np.einsum("bhst,bhtd->bhsd", p, vf)
    wv = wv.transpose(0, 2, 1, 3).reshape(B, S, NS)
    return np.einsum("bsd,ed->bse", wv, np.asarray(w_out, np.float32))


def _build_cmask_host():
    # tri[t, s] = 1.0 if s >= t; column 127 is all-ones (z ones-column).
    t = np.arange(DH)[:, None]
    s = np.arange(DH)[None, :]
    return (s >= t).astype(np.float32)


def _run_on_device(in_maps, trace=False):
    from concourse.bass_utils import run_bass_kernel_spmd

    if "nc" not in _CACHED:
        _CACHED["nc"] = _build_program()
    nc = _CACHED["nc"]
    return run_bass_kernel_spmd(
        nc, in_maps, core_ids=list(range(NCORES)), trace=trace
    )


def _prep_in_maps(input_ids, w_qkv, w_out):
    import ml_dtypes

    bf = ml_dtypes.bfloat16
    x2d = np.ascontiguousarray(
        np.asarray(input_ids, np.float32).reshape(TOK, NS).T
    ).astype(bf)  # [NS, TOK]
    cm = _build_cmask_host().astype(bf)
    wq = np.asarray(w_qkv, np.float32)
    wo = np.asarray(w_out, np.float32)
    in_maps = []
    for c in range(NCORES):
        lo, hi = c * DPC, (c + 1) * DPC
        w_slice = np.concatenate(
            [wq[lo:hi], wq[NS + lo : NS + hi], wq[2 * NS + lo : 2 * NS + hi]],
            axis=0,
        )  # [768, NS] (q,k,v rows for this core's heads)
        wT_c = np.ascontiguousarray(w_slice.T).astype(bf)        # [NS, 768]
        woutT_c = np.ascontiguousarray(wo[:, lo:hi].T).astype(bf)  # [DPC, NS]
        in_maps.append({"xT": x2d, "wT": wT_c, "woutT": woutT_c, "cmask": cm})
    return in_maps


def kernel(input_ids, mask, cache_pos, w_qkv, w_out, k_cache, v_cache):
    if not _causal_fastpath_ok(mask, cache_pos):
        return _numpy_fallback(
            input_ids, mask, cache_pos, w_qkv, w_out, k_cache, v_cache
        )
    in_maps = _prep_in_maps(input_ids, w_qkv, w_out)
    res = _run_on_device(in_maps)
    out = np.zeros((TOK, NS), np.float32)
    for r in res.results:
        out += np.asarray(r["outp"], dtype=np.float32)
    return out.reshape(B, S, NS)



# revision 4
# speedup vs baseline: 1.1275x; 1.1275x over previous
"""Trainium2 Bass kernel for nn_CausalSelfAttention_40810779247124.

Head-sharded (tensor-parallel) causal self-attention prefill across 8
NeuronCores: 2 heads per core.  v2: fp8 DoubleRow projections + free
softmax denominator.

Key ideas vs the bf16 baseline (320.6us):

  * QKV and output projections run as fp8-e4m3 DoubleRow matmuls.  Each
    input is split hi/lo on the host (x = xh + xl exactly at bf16-level
    accuracy) and the product computed with 3 DoubleRow terms
    (xh*wh paired across k-chunks, plus (wh*xl + wl*xh) per chunk):
    25% fewer PE cycles than bf16 at full accuracy.  Inputs are
    pre-scaled (x*8, w*64) to keep the lo residuals out of the fp8
    denormal range; compensation is folded into the exp scale, the
    denominator column, and a host-side divide.
  * Attention keeps scores [t,s] in bf16, but the PV matmul is flipped
    to produce wv^T [s,e] per 128-column quarter.  That makes the
    softmax denominator a free=1 matmul column (1 cycle) instead of a
    free=512 ones-row matmul (a 29us PE saving), at the cost of one
    128x128 PE transpose per quarter (3.4us).  The normalize becomes a
    per-partition tensor_scalar on DVE, and the transposed result is
    split hi/lo into fp8 on ACT+DVE to feed the fp8 out-projection.
  * Causal work is exact at 128-column granularity as in the baseline.

Per-core PE cycles: 295k (QKV) + 70k (scores) + 72k (PV+z) + 8k
(transposes) + 98k (out-proj) ~= 543k ~= 226us vs 733k/306us baseline.

The host verifies mask/cache_pos match causal prefill and falls back to
a numpy reference otherwise.
"""

import sys

sys.path.insert(0, "/opt/trn_rl_repo")

import numpy as np

B = 2
S = 2048
T = 4096
NS = 2048          # n_state
H = 16
DH = 128
NCORES = 8
HPC = H // NCORES  # heads per core = 2
DPC = HPC * DH     # d-slice per core = 256
TOK = B * S        # 4096 tokens across batches
NT = TOK // 512    # 8 token tiles
NK = NS // 128     # 16 contraction chunks
SCALE = 1.0 / float(np.sqrt(DH))

AX = 8.0           # host pre-scale on x
AW = 64.0          # host pre-scale on w_qkv
AO = 64.0          # host pre-scale on w_out
ANWV = 8.0         # on-device scale of normalized wv (via the z column)
# z column value: wv carries AX*AW, so z must carry AX*AW/ANWV for the
# normalized wv to come out scaled by ANWV.
ZCOL = AX * AW / ANWV
# exp( SCALE * q.k ) with q,k carrying AX*AW each
ESCALE = SCALE / (AX * AW) ** 2
# out-projection partials carry ANWV * AO
OUT_SCALE = 1.0 / (ANWV * AO)

_CACHED = {}


def _build_program():
    import concourse.bacc as bacc
    import concourse.bass as bass
    import concourse.tile as tile
    from concourse import mybir

    bf16 = mybir.dt.bfloat16
    f32 = mybir.dt.float32
    fp8 = mybir.dt.float8e4
    DR = mybir.MatmulPerfMode.DoubleRow
    EXP = mybir.ActivationFunctionType.Exp
    COPY = mybir.ActivationFunctionType.Copy
    SUB = mybir.AluOpType.subtract

    nc = bacc.Bacc()

    # x hi/lo fp8, tiled: [NS, tile, (lo,hi), 512]
    xhl = nc.dram_tensor("xhl", [NS, NT, 2, 512], fp8, kind="ExternalInput")
    # w hi/lo fp8: [NS, (hi,lo), 768]  (q0,q1,k0,k1,v0,v1 columns)
    whl = nc.dram_tensor("whl", [NS, 2, 6 * DH], fp8, kind="ExternalInput")
    # w_out hi/lo fp8: [DPC, (hi,lo), NS]
    wouthl = nc.dram_tensor("wouthl", [DPC, 2, NS], fp8, kind="ExternalInput")
    # [tri | identity | zcol]
    cmask = nc.dram_tensor("cmask", [DH, 2 * DH + 1], bf16, kind="ExternalInput")
    outp = nc.dram_tensor("outp", [TOK, NS], bf16, kind="ExternalOutput")

    with tile.TileContext(nc) as tc:
        with (
            tc.tile_pool(name="constp", bufs=1) as constp,
            tc.tile_pool(name="vresp", bufs=1) as vresp,
            tc.tile_pool(name="qkresp", bufs=1) as qkresp,
            tc.tile_pool(name="woutp", bufs=1) as woutp,
        ):
            # tri[t, s] = 1.0 if s >= t; identity for PE transpose; zcol.
            tri2 = constp.tile([DH, 2 * DH + 1], bf16)
            tri = tri2[:, 0:DH]
            ident = tri2[:, DH : 2 * DH]
            zcol = tri2[:, 2 * DH : 2 * DH + 1]

            # V resident: v_res[p, c, e] = V[c*128+p, e] (tok-major)
            v_res = vresp.tile([128, TOK // 128, DPC], bf16)
            # Q,K resident [e-block(q0,q1,k0,k1), tok]
            qk_res = qkresp.tile([128, 4, TOK], bf16)

            # ---------------- phase 1: QKV projection (fp8 DoubleRow) ----
            with (
                tc.tile_pool(name="wp", bufs=1) as wp,
                tc.tile_pool(name="xp", bufs=3) as xp,
                tc.tile_pool(name="qkv_ps", bufs=4, space="PSUM") as qkv_ps,
                tc.tile_pool(name="v_ps", bufs=4, space="PSUM") as v_ps,
            ):
                nc.scalar.dma_start(out=tri2[:, :], in_=cmask[:, :])
                w_sb = wp.tile([128, NK, 2, 6 * DH], fp8)
                wout_sb = woutp.tile([128, HPC, 2, NS], fp8)

                for a in range(NT):
                    pss = [
                        qkv_ps.tile([128, 512], f32, tag="qkv", name=f"qkv{m}")
                        for m in range(4)
                    ]
                    vps = [
                        v_ps.tile([128, 256], f32, tag="vps", name=f"vps{t}")
                        for t in range(4)
                    ]
                    for half in range(2):
                        x_sb = xp.tile([128, NK // 2, 2, 512], fp8, tag="x_sb")
                        for kc in range(NK // 2):
                            kk = half * (NK // 2) + kc
                            nc.sync.dma_start(
                                out=x_sb[:, kc, :, :],
                                in_=xhl[128 * kk : 128 * (kk + 1), a, :, :],
                            )
                        for kc in range(NK // 2):
                            kk = half * (NK // 2) + kc
                            if a == 0:
                                if kk == 0:
                                    for mm in range(6):
                                        nc.scalar.dma_start(
                                            out=w_sb[
                                                :, kk, :, 128 * mm : 128 * (mm + 1)
                                            ],
                                            in_=whl[
                                                128 * kk : 128 * (kk + 1),
                                                :,
                                                128 * mm : 128 * (mm + 1),
                                            ],
                                        )
                                else:
                                    nc.scalar.dma_start(
                                        out=w_sb[:, kk, :, :],
                                        in_=whl[128 * kk : 128 * (kk + 1), :, :],
                                    )
                            last = kk == NK - 1
                            # cross terms: wh*xl + wl*xh for this chunk
                            for m in range(4):
                                nc.tensor.matmul(
                                    pss[m],
                                    w_sb[:, kk, :, 128 * m : 128 * (m + 1)],
                                    x_sb[:, kc, :, :],
                                    start=(kk == 0),
                                    stop=False,
                                    perf_mode=DR,
                                )
                            for t in range(4):
                                nc.tensor.matmul(
                                    vps[t],
                                    x_sb[:, kc, :, 128 * t : 128 * (t + 1)],
                                    w_sb[:, kk, :, 512:768],
                                    start=(kk == 0),
                                    stop=False,
                                    perf_mode=DR,
                                )
                            if kk % 2 == 1:
                                # main terms: wh*xh for chunk pair (kk-1, kk)
                                for m in range(4):
                                    nc.tensor.matmul(
                                        pss[m],
                                        w_sb[:, kk - 1 : kk + 1, 0, 128 * m : 128 * (m + 1)],
                                        x_sb[:, kc - 1 : kc + 1, 1, :],
                                        start=False,
                                        stop=last,
                                        perf_mode=DR,
                                    )
                                for t in range(4):
                                    nc.tensor.matmul(
                                        vps[t],
                                        x_sb[:, kc - 1 : kc + 1, 1, 128 * t : 128 * (t + 1)],
                                        w_sb[:, kk - 1 : kk + 1, 0, 512:768],
                                        start=False,
                                        stop=last,
                                        perf_mode=DR,
                                    )
                    if a < NT - 1:
                        for m in range(4):
                            nc.vector.tensor_copy(
                                out=qk_res[:, m, 512 * a : 512 * (a + 1)],
                                in_=pss[m],
                            )
                        for t in range(4):
                            nc.vector.tensor_copy(
                                out=v_res[:, 4 * a + t, :], in_=vps[t]
                            )
                    else:
                        # Last tile: drain the 8 psum banks across engines so
                        # phase 2 isn't gated on a serial DVE copy chain.
                        for m in range(4):
                            dst = qk_res[:, m, 512 * a : 512 * (a + 1)]
                            if m % 2 == 0:
                                nc.vector.tensor_copy(out=dst, in_=pss[m])
                            else:
                                nc.scalar.activation(
                                    out=dst, in_=pss[m], func=COPY, scale=1.0
                                )
                        for t in range(4):
                            dst = v_res[:, 4 * a + t, :]
                            if t % 2 == 0:
                                nc.vector.tensor_copy(out=dst, in_=vps[t])
                            else:
                                nc.scalar.activation(
                                    out=dst, in_=vps[t], func=COPY, scale=1.0
                                )
                    if a == 0:
                        # Prefetch w_out and warm the ACT exp table while the
                        # PE grinds through the remaining QKV tiles.
                        for h in range(HPC):
                            nc.scalar.dma_start(
                                out=wout_sb[:, h, :, :],
                                in_=wouthl[128 * h : 128 * (h + 1), :, :],
                            )
                        warm = constp.tile([1, 1], f32)
                        nc.scalar.activation(
                            out=warm, in_=tri2[0:1, 0:1], func=EXP, scale=1.0
                        )

            # ------- phases 2+3: attention + out-projection, interleaved ---
            with (
                tc.tile_pool(name="ptp", bufs=20) as ptp,
                tc.tile_pool(name="zrp", bufs=4) as zrp,
                tc.tile_pool(name="nwvp", bufs=6) as nwvp,
                tc.tile_pool(name="wvnp", bufs=2) as wvnp,
                tc.tile_pool(name="ostage", bufs=3) as ostage,
                tc.tile_pool(name="sc_ps", bufs=3, space="PSUM") as sc_ps,
                tc.tile_pool(name="wvq_ps", bufs=2, space="PSUM") as wvq_ps,
                tc.tile_pool(name="tp_ps", bufs=1, space="PSUM") as tp_ps,
                tc.tile_pool(name="o_ps", bufs=2, space="PSUM") as o_ps,
            ):
                # 4 rotating bf16 transpose staging regions (one bank).
                tp_all = tp_ps.tile([128, 4, 128], bf16)

                pending = []  # deferred PE transposes: (nwv, wvn_b, h, scol)
                state = {"tp": 0, "call": 0}

                def flush_tp(n=None):
                    cnt = len(pending) if n is None else min(n, len(pending))
                    for _ in range(cnt):
                        nwv, wvn_b, h, scol = pending.pop(0)
                        r = state["tp"] % 4
                        state["tp"] += 1
                        tps = tp_all[:, r, :]
                        nc.tensor.transpose(tps, nwv, ident)
                        hi = wvn_b[:, h, 1, scol : scol + 128]
                        nc.scalar.activation(
                            out=hi, in_=tps, func=COPY, scale=1.0
                        )
                        nc.vector.tensor_tensor(
                            out=wvn_b[:, h, 0, scol : scol + 128],
                            in0=tps,
                            in1=hi,
                            op=SUB,
                        )

                def attn_tile(b, h, ast, wvn_b):
                    q_sb = qk_res[:, h, S * b + 512 * ast : S * b + 512 * (ast + 1)]
                    nfull = 4 * ast
                    nj = nfull + 4
                    # scores + exp for all chunks of this tile
                    pts = []
                    for j in range(nj):
                        if len(pending) >= 2:
                            flush_tp(1)
                        p = j - nfull
                        lo = 0 if p < 0 else 128 * p  # causal narrowing
                        sc = sc_ps.tile([128, 512], f32, tag="sc")
                        nc.tensor.matmul(
                            sc[:, lo:],
                            qk_res[:, 2 + h, S * b + 128 * j : S * b + 128 * (j + 1)],
                            q_sb[:, lo:],
                            start=True,
                            stop=True,
                        )
                        pt = ptp.tile([128, 512], bf16, tag="pt")
                        nc.scalar.activation(
                            out=pt[:, lo:], in_=sc[:, lo:], func=EXP, scale=ESCALE
                        )
                        if p >= 0:
                            nc.gpsimd.tensor_mul(
                                pt[:, lo : lo + 128], pt[:, lo : lo + 128], tri
                            )
                        pts.append(pt)
                    # PV + denominator, one quarter at a time: each quarter is
                    # a single psum accumulation group over one [128,129] tile
                    # (wv in cols 0..128, z in col 128 — one zero-region).
                    for q in range(4):
                        if len(pending) >= 2:
                            flush_tp(1)
                        qs = slice(128 * q, 128 * (q + 1))
                        njq = nfull + q + 1
                        wz = wvq_ps.tile([128, 129], f32, tag="wvq")
                        for j in range(njq):
                            nc.tensor.matmul(
                                wz[:, 0:128],
                                pts[j][:, qs],
                                v_res[:, 16 * b + j, 128 * h : 128 * (h + 1)],
                                start=(j == 0),
                                stop=False,
                                skip_group_check=True,
                            )
                            nc.tensor.matmul(
                                wz[:, 128:129],
                                pts[j][:, qs],
                                zcol,
                                start=False,
                                stop=(j == njq - 1),
                                skip_group_check=True,
                            )
                        # normalize [s,e] with a per-partition 1/z; transpose
                        # into [e,s] deferred (PE-stall avoidance).
                        zr = zrp.tile([128, 1], f32, tag="zr")
                        nc.vector.reciprocal(out=zr, in_=wz[:, 128:129])
                        nwv = nwvp.tile([128, 128], bf16, tag="nwv")
                        nc.vector.tensor_scalar_mul(nwv, wz[:, 0:128], zr[:, 0:1])
                        pending.append((nwv, wvn_b, h, 512 * ast + 128 * q))

                def outproj(b, ast, wvn_b, last=False):
                    flush_tp()
                    for tk in range(4):
                        toff = 512 * ast + 128 * tk
                        ost = ostage.tile([128, NS], bf16, tag="ost")
                        for n in range(4):
                            # The last job runs with attention done: rotate
                            # through the idle sc banks too for extra depth.
                            if last and n % 2:
                                ops = sc_ps.tile([128, 512], f32, tag="sc")
                            else:
                                ops = o_ps.tile([128, 512], f32, tag="ops")
                            nsl = slice(512 * n, 512 * (n + 1))
                            nc.tensor.matmul(
                                ops,
                                wvn_b[:, 0:2, 1, toff : toff + 128],
                                wout_sb[:, 0:2, 0, nsl],
                                start=True,
                                stop=False,
                                perf_mode=DR,
                            )
                            nc.tensor.matmul(
                                ops,
                                wvn_b[:, 0, 0:2, toff : toff + 128],
                                wout_sb[:, 0, 0:2, nsl],
                                start=False,
                                stop=False,
                                perf_mode=DR,
                            )
                            nc.tensor.matmul(
                                ops,
                                wvn_b[:, 1, 0:2, toff : toff + 128],
                                wout_sb[:, 1, 0:2, nsl],
                                start=False,
                                stop=True,
                                perf_mode=DR,
                            )
                            # Alternate drain engines.
                            dst = ost[:, nsl]
                            if n % 2 == 0:
                                nc.vector.tensor_copy(out=dst, in_=ops)
                            else:
                                nc.scalar.activation(
                                    out=dst, in_=ops, func=COPY, scale=1.0
                                )
                            if n % 2 == 1:
                                nc.sync.dma_start(
                                    out=outp[
                                        S * b + toff : S * b + toff + 128,
                                        1024 * (n // 2) : 1024 * (n // 2 + 1),
                                    ],
                                    in_=ost[:, 1024 * (n // 2) : 1024 * (n // 2 + 1)],
                                )

                jobs = []
                for b in range(B):
                    wvn_b = wvnp.tile(
                        [128, HPC, 2, S], fp8, tag="wvn", name=f"wvn{b}"
                    )
                    for ast in range(4):
                        attn_tile(b, 0, ast, wvn_b)
                        if jobs:
                            outproj(*jobs.pop(0))
                        attn_tile(b, 1, ast, wvn_b)
                        jobs.append((b, ast, wvn_b))
                while jobs:
                    outproj(*jobs.pop(0), last=True)

    nc.compile()
    return nc


def _causal_fastpath_ok(mask, cache_pos):
    if cache_pos.shape != (S,) or not np.array_equal(
        np.asarray(cache_pos), np.arange(S, dtype=np.int64).astype(cache_pos.dtype)
    ):
        return False
    m = np.asarray(mask).reshape(S, T)
    rows = np.arange(S)[:, None]
    cols = np.arange(T)[None, :]
    return np.array_equal(m, cols <= rows)


def _numpy_fallback(input_ids, mask, cache_pos, w_qkv, w_out, k_cache, v_cache):
    x = np.asarray(input_ids, dtype=np.float32)
    qkv = np.einsum("bsd,ed->bse", x, np.asarray(w_qkv, np.float32))
    q, k, v = np.split(qkv, 3, axis=-1)

    def heads(t):
        return t.reshape(B, S, H, DH).transpose(0, 2, 1, 3)

    q, k, v = heads(q), heads(k), heads(v)
    kf = np.array(k_cache, np.float32)
    vf = np.array(v_cache, np.float32)
    kf[:, :, np.asarray(cache_pos)] = k
    vf[:, :, np.asarray(cache_pos)] = v
    sc = np.einsum("bhsd,bhtd->bhst", q, kf) * SCALE
    sc = np.where(np.asarray(mask), sc, np.finfo(np.float32).min)
    sc = sc - sc.max(axis=-1, keepdims=True)
    p = np.exp(sc)
    p = p / p.sum(axis=-1, keepdims=True)
    wv = np.einsum("bhst,bhtd->bhsd", p, vf)
    wv = wv.transpose(0, 2, 1, 3).reshape(B, S, NS)
    return np.einsum("bsd,ed->bse", wv, np.asarray(w_out, np.float32))


def _build_cmask_host():
    # [tri | identity | zcol]: tri[t, s] = 1.0 if s >= t.
    t = np.arange(DH)[:, None]
    s = np.arange(DH)[None, :]
    tri = (s >= t).astype(np.float32)
    ident = np.eye(DH, dtype=np.float32)
    zc = np.full((DH, 1), ZCOL, np.float32)
    return np.concatenate([tri, ident, zc], axis=1)


def _run_on_device(in_maps, trace=False):
    from concourse.bass_utils import run_bass_kernel_spmd

    if "nc" not in _CACHED:
        _CACHED["nc"] = _build_program()
    nc = _CACHED["nc"]
    return run_bass_kernel_spmd(
        nc, in_maps, core_ids=list(range(NCORES)), trace=trace
    )


def _split_hl(arr32):
    """fp8 hi/lo split: arr32 ~= hi + lo with hi,lo e4m3."""
    import ml_dtypes

    f8 = ml_dtypes.float8_e4m3
    hi = arr32.astype(f8)
    lo = (arr32 - hi.astype(np.float32)).astype(f8)
    return hi, lo


def _prep_in_maps(input_ids, w_qkv, w_out):
    import ml_dtypes

    bf = ml_dtypes.bfloat16
    x2d = np.ascontiguousarray(
        np.asarray(input_ids, np.float32).reshape(TOK, NS).T
    ) * AX  # [NS, TOK], pre-scaled
    xh, xl = _split_hl(x2d)
    xhl = np.ascontiguousarray(
        np.stack([xl.reshape(NS, NT, 512), xh.reshape(NS, NT, 512)], axis=2)
    )  # [NS, NT, 2(lo,hi), 512]
    cm = _build_cmask_host().astype(bf)
    wq = np.asarray(w_qkv, np.float32)
    wo = np.asarray(w_out, np.float32)
    in_maps = []
    for c in range(NCORES):
        lo_, hi_ = c * DPC, (c + 1) * DPC
        w_slice = np.concatenate(
            [wq[lo_:hi_], wq[NS + lo_ : NS + hi_], wq[2 * NS + lo_ : 2 * NS + hi_]],
            axis=0,
        )  # [768, NS] (q,k,v rows for this core's heads)
        wT_c = np.ascontiguousarray(w_slice.T) * AW       # [NS, 768]
        wh, wl = _split_hl(wT_c)
        whl_c = np.ascontiguousarray(np.stack([wh, wl], axis=1))  # (hi,lo)
        woT_c = np.ascontiguousarray(wo[:, lo_:hi_].T) * AO  # [DPC, NS]
        woh, wol = _split_hl(woT_c)
        wouthl_c = np.ascontiguousarray(np.stack([woh, wol], axis=1))
        in_maps.append(
            {"xhl": xhl, "whl": whl_c, "wouthl": wouthl_c, "cmask": cm}
        )
    return in_maps


def kernel(input_ids, mask, cache_pos, w_qkv, w_out, k_cache, v_cache):
    if not _causal_fastpath_ok(mask, cache_pos):
        return _numpy_fallback(
            input_ids, mask, cache_pos, w_qkv, w_out, k_cache, v_cache
        )
    in_maps = _prep_in_maps(input_ids, w_qkv, w_out)
    res = _run_on_device(in_maps)
    out = np.zeros((TOK, NS), np.float32)
    for r in res.results:
        out += np.asarray(r["outp"], dtype=np.float32)
    out *= OUT_SCALE
    return out.reshape(B, S, NS)
